# revision 31
# baseline (speedup 1.0000x reference)
"""Trainium2 Bass kernel for nn_Attention_71966472012100.

Multi-head attention, B=4, S=2048, H=12, D=100, HID=1200, bug-faithful
head-mixing reshape before the output projection.

Sharding: 8 cores = batch (4) x head-group (2 groups of 6 heads). Each core
produces 1024 complete rows of the final output; no cross-core comms.

Design (all aimed at keeping PE busy; ~383us/iter vs 530us baseline):
  - V': x^T streamed in 512-col pieces; each 8-bank PSUM group covers
    4 t-tiles x both wv halves so the x stream (0.79us/chunk) stays ahead
    of the PE (1us/chunk); wv loaded chunk-by-chunk alongside.
  - Q/K projected in five M=128 row-groups spanning head boundaries
    (full 128-wide PE array instead of 100/128 per head, -17us); all rows
    spill to DRAM, whose readback re-slices them per head for free, one
    head ahead of the attention consumer.
  - Attention tt loop software-pipelined: PV(tt-2) emitted after scores(tt)
    so PV never waits on the ACT exp latency; ACT exp table pre-warmed.
  - Softmax normalization fully on-chip (no DRAM round-trip): po evicted to
    an f32r scratch, sums row moved from partition 100 to 0 with a tiny
    SBUF->SBUF DMA, DVE reciprocal, ones-matmul broadcast, and the
    normalize multiply fused into the bf16 ot eviction.
  - ot and wo are bf16: halves their SBUF footprint so all of wo stays
    resident, letting WO chains interleave into the attention phase (fills
    the PE bubbles left by the ACT-bound exp stream). A WO slot machine
    emits one chain matmul per scores/PV pair; chains (rt, jb) unlock as
    soon as the heads covering their ot columns are normalized.
  - All DMAs on the SP (sync) HWDGE queue: DMAs on the scalar queue block
    ACT.SEQ between exp issues, and gpsimd SWDGE DMAs raced on HW.
"""

import numpy as np
from contextlib import ExitStack

import ml_dtypes
import concourse.bass as bass
import concourse.tile as tile
from concourse import bacc
from concourse import mybir
from concourse.bass_utils import run_bass_kernel_spmd

F32 = mybir.dt.float32
F32R = mybir.dt.float32r
BF16 = mybir.dt.bfloat16
EXP = mybir.ActivationFunctionType.Exp

B, S, H, D, HID = 4, 2048, 12, 100, 1200
HG = 2                # head groups (tensor parallel)
HL = H // HG          # 6 heads per core
ROWS = S * HL * D // HID   # 1024 output rows per core
CK, CCH = 120, 10     # contraction chunking of HID
TT = S // 128         # 16 key tiles
VW = HL * D + HL      # 606: V' row width per t-tile (d cols + ones col per head)
NM = HID // D         # 12 m-chunks in the output projection

# rt -> last head needed (columns [rt*1536,(rt+1)*1536) of ot, heads are S wide)
RT_LAST_HEAD = [((rt + 1) * 1536 - 1) // S for rt in range(8)]


def _mm(nc, out, lhsT, rhs, **kw):
    nc.tensor.matmul(out, lhsT.bitcast(F32R), rhs.bitcast(F32R), **kw)


def _absorb(nc, ap):
    """PE-side observation of a freshly DMA'd tile (absorbs a DMA wait)."""
    bb = ap.bitcast(BF16)
    nc.tensor.ldweights(bb[:, 0:1])


def build_program(scale: float, n_iters: int = 1):
    nc = bacc.Bacc("TRN2", target_bir_lowering=False, debug=False)

    tn = {}
    tn["xT"] = nc.dram_tensor("xT", [HID, S], F32R, kind="ExternalInput")
    tn["wqT"] = nc.dram_tensor("wqT", [HID, HL * D], F32R, kind="ExternalInput")
    tn["wkT"] = nc.dram_tensor("wkT", [HID, HL * D], F32R, kind="ExternalInput")
    tn["wvT"] = nc.dram_tensor("wvT", [HID, HL * D], F32R, kind="ExternalInput")
    tn["woT"] = nc.dram_tensor("woT", [HID, HID], BF16, kind="ExternalInput")
    tn["biasT"] = nc.dram_tensor("biasT", [128, HL * TT], F32, kind="ExternalInput")
    tn["y"] = nc.dram_tensor("y", [ROWS, HID], F32, kind="ExternalOutput")
    tn["qsp"] = nc.dram_tensor("q_spill", [HL * D, S], F32R)
    tn["ksp"] = nc.dram_tensor("k_spill", [HL * D, S], F32R)

    with tile.TileContext(nc) as tc:
        for _ in range(n_iters):
            _emit_iter(nc, tc, tn, scale)
    nc.compile()
    return nc


def _emit_iter(nc, tc, tn, scale):
    xT, wqT, wkT, wvT, woT = tn["xT"], tn["wqT"], tn["wkT"], tn["wvT"], tn["woT"]
    biasT, y, qsp, ksp = tn["biasT"], tn["y"], tn["qsp"], tn["ksp"]

    with ExitStack() as ctx:
        pa = ctx.enter_context(tc.tile_pool(name="pa", bufs=1))
        vp = pa.tile([128, TT * VW], F32R, name="vp")  # col = tt*VW + h*101 + d
        bias_sb = pa.tile([128, HL * TT], F32, name="bias_sb")
        ones1 = pa.tile([1, D], F32R, name="ones1")
        nc.vector.memset(ones1.bitcast(F32), 1.0)
        nc.vector.tensor_copy(out=ones1, in_=ones1.bitcast(F32))
        # pre-warm the ACT exp table so the first real exp doesn't pay the load
        warm = pa.tile([1, 4], F32R, name="warm")
        nc.scalar.activation(out=warm, in_=ones1[:, 0:4], func=EXP, scale=1.0)

        # rotating q/k SBUF tiles: head 0 written by P1, 1..5 read back
        pqk = ctx.enter_context(tc.tile_pool(name="pqk", bufs=1))
        qt = {}
        kt = {}

        def qk_tiles(h):
            qt[h] = pqk.tile([D, S], F32R, tag="qt", name=f"qt{h}", bufs=2)
            kt[h] = pqk.tile([D, S], F32R, tag="kt", name=f"kt{h}", bufs=2)

        # ================= P1: V' + Q/K projections =======================
        with tc.tile_pool(name="pxt", bufs=1) as pxt, \
             tc.tile_pool(name="pwqk", bufs=1) as pwqk:
            xt = pxt.tile([CK, CCH * S], F32R, name="xt")

            # ones cols pre-set; V cols overwritten
            nc.vector.memset(vp.bitcast(F32), 1.0)
            ones_v = vp.rearrange("p (n k) -> p n k", k=101)[:, :, 100]
            nc.vector.tensor_copy(out=ones_v, in_=ones_v.bitcast(F32))

            def wqk_tiles(gj):
                r0, gm = (0, 128) if gj == 0 else \
                    (gj * 128, 88 if gj == 4 else 128)
                wqh = pwqk.tile([CK, CCH * gm], F32R, tag=f"wq{gm}",
                                name=f"wq{gj}", bufs=2 if gm == 128 else 1)
                wkh = pwqk.tile([CK, CCH * gm], F32R, tag=f"wk{gm}",
                                name=f"wk{gj}", bufs=2 if gm == 128 else 1)
                for wtile, wdram in ((wqh, wqT), (wkh, wkT)):
                    nc.sync.dma_start(
                        out=wtile.rearrange("p (c j) -> p c j", j=gm),
                        in_=wdram.ap()[:, r0 : r0 + gm]
                        .rearrange("(c p) j -> p c j", p=CK))
                return wqh, wkh

            # ---- V' ----
            with tc.tile_pool(name="psv", bufs=8, space="PSUM") as psv, \
                 tc.tile_pool(name="pwv", bufs=1) as pwv:
                # wv bulk, loaded per chunk: col = c*600 + jh*300 + v
                wvf = pwv.tile([CK, CCH * 2 * 300], F32R, name="wvf")
                wqk0 = None
                # 4 groups of 4 t-tiles x both jh halves (8 one-bank accs):
                # each group consumes only 512 xt cols per chunk, so the x
                # stream (0.79us/chunk) stays ahead of the PE (1us/chunk)
                def xt_piece(nc_, g, c):
                    nc_.sync.dma_start(
                        out=xt[:, c * S + g * 512 : c * S + (g + 1) * 512],
                        in_=xT.ap()[c * CK : (c + 1) * CK,
                                    g * 512 : (g + 1) * 512])
                vgroups = [range(0, 4), range(4, 8), range(8, 12),
                           range(12, 14), range(14, 16)]
                for g, tset in enumerate(vgroups):
                    accs = {(i, jh): psv.tile([128, 300], F32, tag="vacc",
                                              name="vacc")
                            for i in tset for jh in range(2)}
                    for c in range(CCH):
                        if g == 0:
                            # own pieces + next group's + the wv chunk
                            xt_piece(nc, 0, c)
                            _absorb(nc, xt[:, c * S : c * S + 512])
                            nc.sync.dma_start(
                                out=wvf[:, c * 600 : (c + 1) * 600],
                                in_=wvT.ap()[c * CK : (c + 1) * CK, :])
                            _absorb(nc, wvf[:, c * 600 : (c + 1) * 600])
                            xt_piece(nc, 1, c)
                        elif g < 3:
                            xt_piece(nc, g + 1, c)
                        if g >= 1:
                            p0, p1 = tset[0] * 128, tset[-1] * 128 + 128
                            _absorb(nc, xt[:, c * S + p0 : c * S + p1])
                        for i in tset:
                            col = c * S + i * 128
                            for jh in range(2):
                                _mm(nc, accs[i, jh][:, :],
                                    xt[:, col : col + 128],
                                    wvf[:, c * 600 + jh * 300 :
                                        c * 600 + (jh + 1) * 300],
                                    start=(c == 0), stop=(c == CCH - 1))
                    if g == 0:
                        # prefetch head 0 projection weights + exp bias
                        wqk0 = wqk_tiles(0)
                        nc.sync.dma_start(out=bias_sb, in_=biasT.ap())
                    for i in tset:
                        for jh in range(2):
                            c0 = i * VW + jh * 3 * 101
                            dst = vp[:, c0 : c0 + 3 * 101].rearrange(
                                "p (hh k) -> p hh k", k=101)[:, :, 0:D]
                            nc.vector.tensor_copy(
                                out=dst,
                                in_=accs[i, jh].rearrange(
                                    "p (hh k) -> p hh k", k=D))

            # ---- Q/K in 5 M-groups of 128 rows spanning head boundaries
            # (full 128-wide array vs 100/128 per-head; the DRAM spill
            # re-slices rows per head for free on readback) ----
            GM = [(0, 128), (128, 128), (256, 128), (384, 128), (512, 88)]
            with tc.tile_pool(name="pstg", bufs=3) as pstg, \
                 tc.tile_pool(name="psq", bufs=3, space="PSUM") as psq:
                for gj, (r0, gm) in enumerate(GM):
                    wqh, wkh = wqk0 if gj == 0 else wqk_tiles(gj)
                    _absorb(nc, wqh)
                    _absorb(nc, wkh)
                    # last group: 1-bank tiles so attention's first ss tiles
                    # alias banks that free earlier
                    qwidth = 512 if gj == len(GM) - 1 else 1024
                    for wtile, dest_dram in ((wqh, qsp), (wkh, ksp)):
                        for q0 in range(0, S, qwidth):
                            acc = psq.tile([gm, qwidth], F32, tag=f"qk{qwidth}",
                                           name="qkacc",
                                           bufs=3 if qwidth == 1024 else 2)
                            for c in range(CCH):
                                for sb in range(qwidth // 512):
                                    s0 = c * S + q0 + sb * 512
                                    _mm(nc, acc[:, sb * 512 : (sb + 1) * 512],
                                        wtile[:, c * gm : (c + 1) * gm],
                                        xt[:, s0 : s0 + 512],
                                        start=(c == 0), stop=(c == CCH - 1))
                            stg = pstg.tile([gm, qwidth], F32R,
                                            tag=f"stg{qwidth}", name="stg")
                            nc.vector.tensor_copy(out=stg, in_=acc[:, :])
                            nc.sync.dma_start(
                                out=dest_dram.ap()[r0 : r0 + gm,
                                                   q0 : q0 + qwidth],
                                in_=stg)
                    if gj == 1:
                        # rows 0-199 spilled: heads 0 and 1 can read back
                        for h in (0, 1):
                            qk_tiles(h)
                            for dst, src in ((qt[h], qsp), (kt[h], ksp)):
                                nc.sync.dma_start(
                                    out=dst,
                                    in_=src.ap()[h * D : (h + 1) * D, :])

        # ================= attention + interleaved WO =====================
        with tc.tile_pool(name="pat", bufs=1, side="right") as pat:
            ot = pat.tile([D, HL * S], BF16, name="ot")
            wob = pat.tile([D, NM * HID], BF16, name="wob")  # col = m*HID + ycol
            for m in range(NM):
                nc.sync.dma_start(
                    out=wob[:, m * HID : (m + 1) * HID],
                    in_=woT.ap()[m * D : (m + 1) * D, :])
            ot_r = ot.rearrange("p (r m) -> p r m", m=NM)

            with tc.tile_pool(name="psa", bufs=2, space="PSUM") as psa, \
                 tc.tile_pool(name="pso", bufs=1, space="PSUM") as pso, \
                 tc.tile_pool(name="psw", bufs=2, space="PSUM") as psw, \
                 tc.tile_pool(name="ppt", bufs=3) as ppt, \
                 tc.tile_pool(name="pnr", bufs=2) as pnr, \
                 tc.tile_pool(name="por", bufs=2) as por, \
                 tc.tile_pool(name="pyb", bufs=3) as pyb:

                # WO slot machine: one matmul per call, interleaved into the
                # attention tt loops so the PE always has exp-independent work
                wo_st = {"queue": [], "chain": None, "m": 0, "pys": None,
                         "tag": "pys"}

                def wo_slot(n=1):
                    for _ in range(n):
                        if wo_st["chain"] is None:
                            if not wo_st["queue"]:
                                return
                            wo_st["chain"] = wo_st["queue"].pop(0)
                            wo_st["m"] = 0
                            wo_st["pys"] = psw.tile([128, 512], F32,
                                                    tag=wo_st["tag"],
                                                    name="pys", bufs=1)
                        rt, jb = wo_st["chain"]
                        m = wo_st["m"]
                        nc.tensor.matmul(
                            wo_st["pys"][:, 0:400],
                            ot_r[:, rt * 128 : (rt + 1) * 128, m],
                            wob[:, m * HID + jb * 400 :
                                m * HID + (jb + 1) * 400],
                            start=(m == 0), stop=(m == NM - 1))
                        wo_st["m"] += 1
                        if wo_st["m"] == NM:
                            ysb = pyb.tile([128, 400], F32, tag="ysb",
                                           name="ysb")
                            if wo_st.get("use_act"):
                                # tail: ACT engine + queue are idle by now
                                nc.scalar.copy(out=ysb,
                                               in_=wo_st["pys"][:, 0:400])
                                nc.scalar.dma_start(
                                    out=y.ap()[rt * 128 : (rt + 1) * 128,
                                               jb * 400 : (jb + 1) * 400],
                                    in_=ysb)
                            else:
                                nc.vector.tensor_copy(
                                    out=ysb, in_=wo_st["pys"][:, 0:400])
                                nc.sync.dma_start(
                                    out=y.ap()[rt * 128 : (rt + 1) * 128,
                                               jb * 400 : (jb + 1) * 400],
                                    in_=ysb)
                            wo_st["chain"] = None

                for h in range(HL):
                    if h >= 1:
                        # absorb the DMA waits of this head's prefetched q/k
                        _absorb(nc, qt[h])
                        _absorb(nc, kt[h])
                    if 1 <= h + 1 < HL:
                        # prefetch next head's q/k from DRAM spill
                        qk_tiles(h + 1)
                        for dst, src in ((qt[h + 1], qsp), (kt[h + 1], ksp)):
                            nc.sync.dma_start(
                                out=dst,
                                in_=src.ap()[(h + 1) * D : (h + 2) * D, :])
                    for sh in range(2):
                        s0 = sh * 1024
                        po = pso.tile([D + 1, 1024], F32, tag="po", name="po")
                        # software-pipelined: PV(tt-3) emitted after scores(tt)
                        LAG = 3
                        pts = [None] * TT
                        for tt in range(TT + LAG):
                            if tt < TT:
                                ss = psa.tile([128, 1024], F32, tag="ss",
                                              name="ss")
                                for sbb in range(2):
                                    _mm(nc, ss[:, sbb * 512 : (sbb + 1) * 512],
                                        kt[h][:, tt * 128 : (tt + 1) * 128],
                                        qt[h][:, s0 + sbb * 512 :
                                              s0 + (sbb + 1) * 512],
                                        start=True, stop=True)
                                wo_slot(1)
                                pt = ppt.tile([128, 1024], F32R, tag="pt",
                                              name="pt", bufs=LAG + 2)
                                nc.scalar.activation(
                                    out=pt, in_=ss[:, :], func=EXP,
                                    bias=bias_sb[:, h * TT + tt :
                                                 h * TT + tt + 1],
                                    scale=scale)
                                pts[tt] = pt
                            if tt >= LAG:
                                for sbb in range(2):
                                    _mm(nc, po[:, sbb * 512 : (sbb + 1) * 512],
                                        vp[:, (tt - LAG) * VW + h * 101 :
                                           (tt - LAG) * VW + h * 101 + 101],
                                        pts[tt - LAG][:, sbb * 512 :
                                                      (sbb + 1) * 512],
                                        start=(tt == LAG),
                                        stop=(tt == TT + LAG - 1))
                                pts[tt - LAG] = None
                                wo_slot(1)
                        # ---- on-chip normalization + eviction to ot ----
                        orow = por.tile([D + 1, 1024], F32R, tag="orow",
                                        name="orow")
                        nc.vector.tensor_copy(out=orow, in_=po[:, :])
                        # sums row (partition 100) -> partition 0 via DMA
                        srow = pnr.tile([1, 1024], F32R, tag="srow",
                                        name="srow")
                        rrow = pnr.tile([1, 1024], F32R, tag="rrow", name="rrow")
                        nc.sync.dma_start(out=srow, in_=orow[D : D + 1, :])
                        with nc.allow_low_precision(reason="softmax recip"):
                            nc.vector.reciprocal(out=rrow, in_=srow)
                        for blk in range(2):
                            pb = psw.tile([128, 512], F32, tag="nrm",
                                          name="pb", bufs=1)
                            _mm(nc, pb[0:D, :], ones1[0:1, :],
                                rrow[0:1, blk * 512 : (blk + 1) * 512],
                                start=True, stop=True)
                            nc.vector.tensor_mul(
                                ot[:, h * S + s0 + blk * 512 :
                                   h * S + s0 + (blk + 1) * 512],
                                orow[0:D, blk * 512 : (blk + 1) * 512],
                                pb[0:D, :])
                    # head h normalized -> unlock rts
                    for rt in range(8):
                        if RT_LAST_HEAD[rt] == h:
                            for jb in range(3):
                                wo_st["queue"].append((rt, jb))
                wo_st["use_act"] = True
                while wo_st["queue"] or wo_st["chain"] is not None:
                    wo_slot(1)


def make_core_inputs(x, alibi, attention_mask, wq, wk, wv, wo, layer_index):
    li = int(np.asarray(layer_index))
    inv = np.float32(1.0 / (li + 1))
    woT = np.ascontiguousarray(
        np.asarray(wo, dtype=np.float32).T).astype(ml_dtypes.bfloat16)
    xTs = [np.ascontiguousarray(np.asarray(x[b], dtype=np.float32).T)
           for b in range(B)]
    wts = []
    for g in range(HG):
        sl = slice(g * HL * D, (g + 1) * HL * D)
        wts.append(tuple(
            np.ascontiguousarray(np.asarray(w, dtype=np.float32)[sl, :].T)
            for w in (wq, wk, wv)))
    in_maps = []
    for b in range(B):
        for g in range(HG):
            a = np.asarray(alibi, dtype=np.float32)[
                b * H + g * HL : b * H + (g + 1) * HL, 0, :]      # (6, S)
            msk = np.asarray(attention_mask, dtype=np.float32)[b, 0, 0, :S]
            bias = a * inv + msk[None, :]                          # (6, S)
            biasT = np.ascontiguousarray(
                bias.reshape(HL, TT, 128).transpose(2, 0, 1).reshape(128, HL * TT))
            wqT, wkT, wvT = wts[g]
            in_maps.append({
                "xT": xTs[b], "wqT": wqT, "wkT": wkT, "wvT": wvT,
                "woT": woT, "biasT": biasT,
            })
    scale = float(np.float32(np.sqrt(np.float32(D))) * inv)
    return in_maps, scale


def run(trace=False, **inputs):
    in_maps, scale = make_core_inputs(**inputs)
    nc = build_program(scale)
    res = run_bass_kernel_spmd(nc, in_maps, core_ids=list(range(B * HG)),
                               trace=trace)
    out = np.empty((B, S, HID), dtype=np.float32)
    for b in range(B):
        for g in range(HG):
            out[b, g * ROWS : (g + 1) * ROWS, :] = res.results[b * HG + g]["y"]
    return out, res


def kernel(**inputs) -> np.ndarray:
    out, _ = run(trace=False, **inputs)
    return out
